# revision 18
# baseline (speedup 1.0000x reference)
"""Trainium2 Bass kernel for ContextEncodingTransformer (layer_id==1 path).

Data-parallel over BT across 8 NeuronCores: core i handles image batches
8i..8i+8 (= output rows 96i..96i+96).

Per-core dataflow (all matmuls in float32r, fp32 PSUM accumulate):
  roi conv1x1   : conv_roi[c,(r,p)] = w_ds1T.T @ roi_t          (K=d, 8 chunks)
  emb           : emb_rc[r,o]      += conv_roi[:,:,p].T @ w_embT[:,p,:]
  emb^T         : PE transpose -> embT[c,r]  (score stationary)
  per image b:
    img conv1x1 : img[c,s] = w_ds2T.T @ raw                      (K=C, 6 chunks)
    combo       : img_chunk.T @ [I_128 | embT_b] -> [imgT tile | a^T cols]
                  (one matmul transposes img AND computes scores)
    exp         : E^T = exp(a^T - SHIFT)   (global shift; softmax-invariant)
    ctx         : ctx[n, 0:256]+rowsum = E^T.T @ [imgT | ones]   (K=s, 29 chunks)
    scale       : ctx *= 1/rowsum
  tail          : LN1(ctx+emb) -> x; x^T; FFN (relu) in [c,r]; (ffn)^T;
                  LN2(x+ffn) -> out[96, 256]
"""

import os
import sys
import types

import numpy as np

sys.path.insert(0, "/opt/trn_rl_repo")

import concourse.bass as bass
import concourse.tile as tile
from concourse import bacc, mybir
from concourse.bass_utils import run_bass_kernel_spmd
from concourse.masks import make_identity

F32 = mybir.dt.float32
F32R = mybir.dt.float32r

N_CORES = 8
NFC = 256
D = 1024
KK = 25          # 5x5 kernel positions
NROI = 12        # rois per image
B = 8            # images per core
R = 96           # rows per core (B * NROI)
S = 3600         # 45*80 spatial
CIMG = 768
SHIFT = 60.0     # global softmax shift (max logit ~85; see notes)
EPS = 1e-5

NK = 29          # s-chunks of 128 (28*128 + 16)
SK_LAST = S - 128 * (NK - 1)

LAST_RESULTS = None  # BassKernelResults of the most recent run (for test.py)


def _r(x):
    return x


def build_bass():
    nc = bacc.Bacc("TRN2", target_bir_lowering=False, debug=False,
                   num_devices=N_CORES)

    # ---- DRAM I/O (per-core shard) ----
    roi_d = nc.dram_tensor("roi_t", [D, R * KK], F32R, kind="ExternalInput").ap()
    img_d = nc.dram_tensor("img_raw", [B, CIMG, S], F32R, kind="ExternalInput").ap()
    w1_d = nc.dram_tensor("w_ds1t", [D, NFC], F32R, kind="ExternalInput").ap()
    w2_d = nc.dram_tensor("w_ds2t", [CIMG, NFC], F32R, kind="ExternalInput").ap()
    we_d = nc.dram_tensor("w_embt", [NFC, KK * NFC], F32R, kind="ExternalInput").ap()
    b1_d = nc.dram_tensor("b_ds1", [NFC], F32, kind="ExternalInput").ap()
    b2_d = nc.dram_tensor("b_ds2", [NFC], F32, kind="ExternalInput").ap()
    be_d = nc.dram_tensor("b_emb", [NFC], F32, kind="ExternalInput").ap()
    fw1_d = nc.dram_tensor("ffn_w1t", [NFC, NFC], F32R, kind="ExternalInput").ap()
    fb1_d = nc.dram_tensor("ffn_b1", [NFC], F32, kind="ExternalInput").ap()
    fw2_d = nc.dram_tensor("ffn_w2t", [NFC, NFC], F32R, kind="ExternalInput").ap()
    fb2_d = nc.dram_tensor("ffn_b2", [NFC], F32, kind="ExternalInput").ap()
    g1_d = nc.dram_tensor("g1", [NFC], F32, kind="ExternalInput").ap()
    be1_d = nc.dram_tensor("be1", [NFC], F32, kind="ExternalInput").ap()
    g2_d = nc.dram_tensor("g2", [NFC], F32, kind="ExternalInput").ap()
    be2_d = nc.dram_tensor("be2", [NFC], F32, kind="ExternalInput").ap()
    out_d = nc.dram_tensor("out", [R, NFC], F32, kind="ExternalOutput").ap()

    with tile.TileContext(nc) as tc:
        _body(nc, tc, roi_d, img_d, w1_d, w2_d, we_d, b1_d, b2_d, be_d,
              fw1_d, fb1_d, fw2_d, fb2_d, g1_d, be1_d, g2_d, be2_d, out_d)

    nc.compile()
    return nc


SECS = [(0, 1152), (1152, 1152), (2304, 1152), (3456, 144)]


def _body(nc, tc, roi_d, img_d, w1_d, w2_d, we_d, b1_d, b2_d, be_d,
          fw1_d, fb1_d, fw2_d, fb2_d, g1_d, be1_d, g2_d, be2_d, out_d):
    from contextlib import ExitStack

    top = ExitStack()
    persist = top.enter_context(tc.tile_pool(name="persist", bufs=1))
    ip_w = top.enter_context(tc.tile_pool(name="iw", bufs=1))
    ip_raw = top.enter_context(tc.tile_pool(name="raw", bufs=2))
    ip_img = top.enter_context(tc.tile_pool(name="img", bufs=3))
    ps_main = top.enter_context(tc.tile_pool(name="ps_main", bufs=4,
                                             space="PSUM"))
    ps_combo = top.enter_context(tc.tile_pool(name="ps_combo", bufs=2,
                                              space="PSUM"))
    ps_ctx = top.enter_context(tc.tile_pool(name="ps_ctx", bufs=1,
                                            space="PSUM"))
    ps_emb = top.enter_context(tc.tile_pool(name="ps_emb", bufs=1,
                                            space="PSUM"))

    # --- weight / activation loads (issue order == DMA priority) ---
    w2sb = ip_w.tile([128, 6, NFC], F32R)
    nc.sync.dma_start(out=w2sb[:],
                      in_=w2_d.rearrange("(k p) o -> p k o", p=128))

    def load_raw(b, si):
        s0, sl = SECS[si]
        raw = ip_raw.tile([128, 6, 1152], F32R, tag="raw",
                          name=f"raw_{b}_{si}")
        nc.sync.dma_start(
            out=raw[:, :, :sl],
            in_=img_d[b].rearrange("(k p) s -> p k s", p=128)[
                :, :, s0:s0 + sl])
        return raw

    raw_pre = [load_raw(0, 0), load_raw(0, 1)]

    ident = persist.tile([128, 128], F32)
    make_identity(nc, ident[:])
    b2_sb = persist.tile([128, 2], F32)
    nc.sync.dma_start(out=b2_sb[:], in_=b2_d.rearrange("(c p) -> p c", p=128))
    b1_sb = persist.tile([128, 2], F32)
    nc.sync.dma_start(out=b1_sb[:], in_=b1_d.rearrange("(c p) -> p c", p=128))
    bemb_bc = persist.tile([R, NFC], F32)
    nc.sync.dma_start(out=bemb_bc[:],
                      in_=be_d[None, :].to_broadcast((R, NFC)))
    ones58 = persist.tile([128, 58], F32)
    nc.vector.memset(ones58[:], 1.0)
    nshift_sb = persist.tile([128, 1], F32)
    nc.vector.memset(nshift_sb[:], -SHIFT)
    eps_sb = persist.tile([128, 1], F32)
    nc.vector.memset(eps_sb[:], EPS)

    emb_rc = persist.tile([R, NFC], F32)      # emb in [row, o]
    embT = persist.tile([128, 2, R], F32)     # emb^T  [c_part, c_half, row]
    ctx_sb = persist.tile([R, NFC], F32)      # attention context, packed rows

    img_secs = {}

    def conv_sec(b, si):
        """img conv for section si of batch b -> img_secs[(b, si)]."""
        s0, sl = SECS[si]
        raw = raw_pre.pop(0) if raw_pre else load_raw(b, si)
        img_sb = ip_img.tile([128, 2, 1152], F32R, tag="img",
                             name=f"img_{b}_{si}")
        img_secs[(b, si)] = img_sb
        nf = 3 if sl == 1152 else 1
        fl = sl // nf
        for ch in range(2):
            pss = {}
            for Ck in range(6):
                for f in range(nf):
                    if Ck == 0:
                        pss[f] = ps_main.tile([128, 384], F32, tag="conv",
                                              name=f"cv_{b}_{si}_{ch}_{f}")
                    nc.tensor.matmul(
                        pss[f][:, :fl],
                        w2sb[:, Ck, ch * 128:(ch + 1) * 128],
                        raw[:, Ck, f * fl:(f + 1) * fl],
                        start=(Ck == 0), stop=(Ck == 5))
            for f in range(nf):
                nc.scalar.activation(
                    out=img_sb[:, ch, f * fl:(f + 1) * fl],
                    in_=pss[f][:, :fl],
                    func=mybir.ActivationFunctionType.Identity,
                    bias=b2_sb[:, ch:ch + 1], scale=1.0)

    # ==================== roi phase (streamed) ====================
    roi_scope = ExitStack()
    rp = roi_scope.enter_context(tc.tile_pool(name="roi", bufs=1))
    rpf = roi_scope.enter_context(tc.tile_pool(name="roi_f", bufs=2))
    rpg = roi_scope.enter_context(tc.tile_pool(name="roi_pg", bufs=2))

    def roi_phase_conv():
        w1sb = rp.tile([128, 8, NFC], F32R)
        nc.sync.dma_start(out=w1sb[:],
                          in_=w1_d.rearrange("(k p) o -> p k o", p=128))
        conv_roi = rp.tile([128, 2, R, KK], F32R)  # [c_part, c_half, r, p]
        roi_r = roi_d.rearrange("(k p) f -> p k f", p=128)
        for f in range(5):
            rsf = rpf.tile([128, 8, 480], F32R, tag="roisb",
                           name=f"roisb_{f}")
            nc.sync.dma_start(out=rsf[:], in_=roi_r[:, :, f * 480:(f + 1) * 480])
            for ch in range(2):
                ps = ps_main.tile([128, 480], F32, tag="conv",
                                  name=f"rcv_{ch}_{f}")
                for dk in range(8):
                    nc.tensor.matmul(
                        ps[:],
                        w1sb[:, dk, ch * 128:(ch + 1) * 128],
                        rsf[:, dk, :],
                        start=(dk == 0), stop=(dk == 7))
                nc.scalar.activation(
                    out=conv_roi[:, ch].rearrange("p r q -> p (r q)")[
                        :, f * 480:(f + 1) * 480],
                    in_=ps[:],
                    func=mybir.ActivationFunctionType.Identity,
                    bias=b1_sb[:, ch:ch + 1], scale=1.0)
        return conv_roi

    def roi_phase_emb(conv_roi):
        we_r = we_d.rearrange("(k p) f -> p k f", p=128)
        eps_ps = ps_emb.tile([R, NFC], F32, tag="emb")
        for pg in range(5):
            wes = rpg.tile([128, 2, 5, NFC], F32R, tag="wesb",
                           name=f"wesb_{pg}")
            nc.sync.dma_start(
                out=wes[:],
                in_=we_r.rearrange("p k (q o) -> p k q o", o=NFC)[
                    :, :, pg * 5:(pg + 1) * 5, :])
            for ck in range(2):
                for pl in range(5):
                    p = pg * 5 + pl
                    nc.tensor.matmul(
                        eps_ps[:],
                        conv_roi[:, ck, :, p],
                        wes[:, ck, pl, :],
                        start=(pg == 0 and ck == 0 and pl == 0),
                        stop=(pg == 4 and ck == 1 and pl == 4))
        nc.vector.tensor_add(emb_rc[:], eps_ps[:], bemb_bc[:])
        for ch in range(2):
            tps_e = ps_ctx.tile([128, R], F32, tag="ctx", name=f"embt_{ch}")
            nc.tensor.transpose(tps_e[:, :R],
                                emb_rc[:, ch * 128:(ch + 1) * 128],
                                ident[:R, :R])
            nc.vector.tensor_copy(embT[:, ch, :], tps_e[:, :R])

    # ==================== emission ====================
    conv_sec(0, 0)
    conv_sec(0, 1)
    conv_roi = roi_phase_conv()
    conv_sec(0, 2)
    roi_phase_emb(conv_roi)
    conv_sec(0, 3)
    roi_scope.close()

    # post-roi pools
    ip_imgt = top.enter_context(tc.tile_pool(name="imgt", bufs=2))
    ip_small = top.enter_context(tc.tile_pool(name="ismall", bufs=2))
    rhs_sc = [ip_w.tile([128, 140], F32R, tag=f"rhs_sc{ch}",
                        name=f"rhs_sc{ch}")
              for ch in range(2)]
    for ch in range(2):
        nc.vector.tensor_copy(rhs_sc[ch][:, 0:128], ident[:])

    def combo_sec(b, si, cps):
        """transpose+score+exp+ctx accumulation for section si of batch b."""
        img_sb = img_secs.pop((b, si))
        s0, sl = SECS[si]
        k0 = s0 // 128
        nk = (sl + 127) // 128
        imgt = ip_imgt.tile([128, 9, NFC + 2], F32R, tag="imgt",
                            name=f"imgt_{b}_{si}")
        et = ip_small.tile([128, 9, NROI], F32R, tag="et", name=f"et_{b}_{si}")
        nc.vector.tensor_copy(
            imgt[:, :nk, NFC:NFC + 2],
            ones58[:, :2 * nk].rearrange("p (k two) -> p k two", two=2))
        for j in range(nk):
            sk = min(128, sl - j * 128)
            pt = []
            for ch in range(2):
                ps = ps_combo.tile([128, 140], F32, tag="combo")
                nc.tensor.matmul(
                    ps[:sk, :],
                    img_sb[:, ch, j * 128:j * 128 + sk],
                    rhs_sc[ch][:],
                    start=True, stop=True)
                nc.vector.tensor_copy(
                    imgt[:sk, j, ch * 128:(ch + 1) * 128], ps[:sk, 0:128])
                pt.append(ps)
            aT = ip_small.tile([128, NROI], F32, tag="aT")
            nc.vector.tensor_copy(aT[:sk, :], pt[0][:sk, 128:140])
            aT2 = ip_small.tile([128, NROI], F32, tag="aT2")
            nc.vector.tensor_add(aT2[:sk, :], pt[1][:sk, 128:140], aT[:sk, :])
            nc.scalar.activation(
                out=et[:sk, j, :], in_=aT2[:sk, :],
                func=mybir.ActivationFunctionType.Exp,
                bias=nshift_sb[:sk, :], scale=1.0)
        for j in range(nk):
            sk = min(128, sl - j * 128)
            k = k0 + j
            nc.tensor.matmul(
                cps[:], et[:sk, j, :], imgt[:sk, j, :],
                start=(k == 0), stop=(k == NK - 1))

    def ctx_finish(b, cps):
        rinv = ip_small.tile([NROI, 1], F32, tag="rinv")
        nc.vector.reciprocal(rinv[:], cps[:, NFC:NFC + 1])
        ctx_b = ip_small.tile([NROI, NFC], F32, tag="ctx_b")
        nc.vector.tensor_scalar_mul(ctx_b[:], cps[:, 0:NFC], rinv[:])
        nc.sync.dma_start(out=ctx_sb[b * NROI:(b + 1) * NROI, :],
                          in_=ctx_b[:])

    for b in range(B):
        for ch in range(2):
            nc.vector.tensor_copy(
                rhs_sc[ch][:, 128:140],
                embT[:, ch, b * NROI:(b + 1) * NROI])
        cps = ps_ctx.tile([NROI, NFC + 2], F32, tag="ctx", name=f"ctx_{b}")
        for si in range(4):
            if b > 0:
                conv_sec(b, si)
            combo_sec(b, si, cps)
        ctx_finish(b, cps)

    # ==================== tail: LN1 -> FFN -> LN2 ====================
    with ExitStack() as t_ctx:
        tp = t_ctx.enter_context(tc.tile_pool(name="tail", bufs=1))

        def bcast(src, name):
            t = tp.tile([R, NFC], F32, name=name)
            nc.sync.dma_start(out=t[:], in_=src[None, :].to_broadcast((R, NFC)))
            return t

        g1_bc = bcast(g1_d, "g1bc")
        be1_bc = bcast(be1_d, "be1bc")
        g2_bc = bcast(g2_d, "g2bc")
        be2_bc = bcast(be2_d, "be2bc")
        fb1_sb = tp.tile([128, 2], F32)
        nc.sync.dma_start(out=fb1_sb[:],
                          in_=fb1_d.rearrange("(c p) -> p c", p=128))
        fb2_sb = tp.tile([128, 2], F32)
        nc.sync.dma_start(out=fb2_sb[:],
                          in_=fb2_d.rearrange("(c p) -> p c", p=128))
        fw1 = tp.tile([128, 2, NFC], F32R)
        nc.sync.dma_start(out=fw1[:],
                          in_=fw1_d.rearrange("(k p) o -> p k o", p=128))
        fw2 = tp.tile([128, 2, NFC], F32R)
        nc.sync.dma_start(out=fw2[:],
                          in_=fw2_d.rearrange("(k p) o -> p k o", p=128))

        def layernorm(dst, src, g_bc, b_bc):
            st = tp.tile([R, 6], F32, tag="ln_st")
            mv = tp.tile([R, 2], F32, tag="ln_mv")
            nc.vector.bn_stats(out=st[:], in_=src[:])
            nc.vector.bn_aggr(out=mv[:], in_=st[:])
            rstd = tp.tile([R, 1], F32, tag="ln_rstd")
            nc.scalar.activation(out=rstd[:], in_=mv[:, 1:2],
                                 func=mybir.ActivationFunctionType.Sqrt,
                                 bias=eps_sb[:R, :], scale=1.0)
            nc.vector.reciprocal(rstd[:], rstd[:])
            nc.vector.tensor_scalar(
                out=dst[:], in0=src[:], scalar1=mv[:, 0:1], scalar2=rstd[:],
                op0=mybir.AluOpType.subtract, op1=mybir.AluOpType.mult)
            nc.vector.tensor_mul(dst[:], dst[:], g_bc[:])
            nc.vector.tensor_add(dst[:], dst[:], b_bc[:])

        x_in = tp.tile([R, NFC], F32)
        nc.vector.tensor_add(x_in[:], ctx_sb[:], emb_rc[:])
        x = tp.tile([R, NFC], F32)
        layernorm(x, x_in, g1_bc, be1_bc)

        xT = tp.tile([128, 2, R], F32R)
        for ch in range(2):
            ps = ps_ctx.tile([128, R], F32, tag="ctx", name=f"xT_{ch}")
            nc.tensor.transpose(ps[:, :R], x[:, ch * 128:(ch + 1) * 128],
                                ident[:R, :R])
            nc.vector.tensor_copy(xT[:, ch, :], ps[:, :R])

        h = tp.tile([128, 2, R], F32R)
        for ch in range(2):
            ps = ps_combo.tile([128, R], F32, tag="combo", name=f"ffn1_{ch}")
            for ck in range(2):
                nc.tensor.matmul(ps[:, :R],
                                 fw1[:, ck, ch * 128:(ch + 1) * 128],
                                 xT[:, ck, :],
                                 start=(ck == 0), stop=(ck == 1))
            nc.scalar.activation(out=h[:, ch, :], in_=ps[:, :R],
                                 func=mybir.ActivationFunctionType.Relu,
                                 bias=fb1_sb[:, ch:ch + 1], scale=1.0)

        f_rc = tp.tile([R, NFC], F32)
        for ch in range(2):
            ps = ps_combo.tile([128, R], F32, tag="combo", name=f"ffn2_{ch}")
            for ck in range(2):
                nc.tensor.matmul(ps[:, :R],
                                 fw2[:, ck, ch * 128:(ch + 1) * 128],
                                 h[:, ck, :],
                                 start=(ck == 0), stop=(ck == 1))
            fo = tp.tile([128, R], F32, tag="ffn_o")
            nc.scalar.activation(out=fo[:], in_=ps[:, :R],
                                 func=mybir.ActivationFunctionType.Identity,
                                 bias=fb2_sb[:, ch:ch + 1], scale=1.0)
            pst = ps_ctx.tile([R, 128], F32, tag="ctx", name=f"fT_{ch}")
            nc.tensor.transpose(pst[:], fo[:], ident[:])
            nc.vector.tensor_copy(f_rc[:, ch * 128:(ch + 1) * 128], pst[:])

        x2 = tp.tile([R, NFC], F32)
        nc.vector.tensor_add(x2[:], x[:], f_rc[:])
        out_sb = tp.tile([R, NFC], F32)
        layernorm(out_sb, x2, g2_bc, be2_bc)
        nc.sync.dma_start(out=out_d[:], in_=out_sb[:])

    top.close()


_NC_CACHE = None


def _get_nc():
    global _NC_CACHE
    if _NC_CACHE is None:
        _NC_CACHE = build_bass()
    return _NC_CACHE


def _install_ntff_hook():
    """The image's antenv lacks axon_hooks; register the NTFF profile hook."""
    if "antenv.axon_hooks" in sys.modules:
        return
    try:
        sys.path.insert(0, "/root/.axon_site/trn_agent_boot")
        import trn_boot
        hook = trn_boot._ntff_profile_via_ctypes("/opt/axon/libaxon_pjrt.so")
        m = types.ModuleType("antenv.axon_hooks")
        m.get_axon_ntff_profile_hook = lambda: hook
        sys.modules["antenv.axon_hooks"] = m
    except Exception:
        pass


def prepare_in_maps(roi_feature, image_feature, w_ds1, b_ds1, w_ds2, b_ds2,
                    w_emb, b_emb, g1, be1, ffn_w1, ffn_b1, ffn_w2, ffn_b2,
                    g2, be2, layer_id=1, **_unused):
    roi_feature = np.ascontiguousarray(np.asarray(roi_feature, dtype=np.float32))
    image_feature = np.ascontiguousarray(np.asarray(image_feature, dtype=np.float32))

    f32 = lambda x: np.ascontiguousarray(np.asarray(x, dtype=np.float32))
    w_ds1t = f32(w_ds1).T.copy()                       # [D, NFC]
    w_ds2t = f32(w_ds2).T.copy()                       # [CIMG, NFC]
    w_embt = np.ascontiguousarray(
        f32(w_emb).reshape(NFC, NFC, KK).transpose(1, 2, 0).reshape(
            NFC, KK * NFC))                            # [c, p*o]
    ffn_w1t = f32(ffn_w1).T.copy()
    ffn_w2t = f32(ffn_w2).T.copy()

    shared = {
        "w_ds1t": w_ds1t, "w_ds2t": w_ds2t, "w_embt": w_embt,
        "b_ds1": f32(b_ds1), "b_ds2": f32(b_ds2), "b_emb": f32(b_emb),
        "ffn_w1t": ffn_w1t, "ffn_b1": f32(ffn_b1),
        "ffn_w2t": ffn_w2t, "ffn_b2": f32(ffn_b2),
        "g1": f32(g1), "be1": f32(be1), "g2": f32(g2), "be2": f32(be2),
    }

    roi_r = roi_feature.reshape(N_CORES, R, D, KK)
    img_r = image_feature.reshape(N_CORES, B, CIMG, S)
    in_maps = []
    for i in range(N_CORES):
        roi_t = np.ascontiguousarray(
            roi_r[i].transpose(1, 0, 2)).reshape(D, R * KK)
        in_maps.append({"roi_t": roi_t,
                        "img_raw": np.ascontiguousarray(img_r[i]),
                        **shared})
    return in_maps


def kernel(**inputs):
    global LAST_RESULTS
    in_maps = prepare_in_maps(**inputs)
    nc = _get_nc()
    trace = os.environ.get("BASS_KERNEL_TRACE", "0") == "1"
    if trace:
        _install_ntff_hook()
    LAST_RESULTS = run_bass_kernel_spmd(
        nc, in_maps, list(range(N_CORES)), trace=trace)
    out = np.concatenate([LAST_RESULTS.results[i]["out"]
                          for i in range(N_CORES)], axis=0)
    return out


# revision 19
# speedup vs baseline: 1.1113x; 1.1113x over previous
"""Trainium2 Bass kernel for ContextEncodingTransformer (layer_id==1 path).

Data-parallel over BT across 8 NeuronCores: core i handles image batches
8i..8i+8 (= output rows 96i..96i+96).

Per-core dataflow (all matmuls in float32r, fp32 PSUM accumulate):
  roi conv1x1   : conv_roi[c,(r,p)] = w_ds1T.T @ roi_t          (K=d, 8 chunks)
  emb           : emb_rc[r,o]      += conv_roi[:,:,p].T @ w_embT[:,p,:]
  emb^T         : PE transpose -> embT[c,r]  (score stationary)
  per image b:
    img conv1x1 : img[c,s] = w_ds2T.T @ raw                      (K=C, 6 chunks)
    combo       : img_chunk.T @ [I_128 | embT_b] -> [imgT tile | a^T cols]
                  (one matmul transposes img AND computes scores)
    exp         : E^T = exp(a^T - SHIFT)   (global shift; softmax-invariant)
    ctx         : ctx[n, 0:256]+rowsum = E^T.T @ [imgT | ones]   (K=s, 29 chunks)
    scale       : ctx *= 1/rowsum
  tail          : LN1(ctx+emb) -> x; x^T; FFN (relu) in [c,r]; (ffn)^T;
                  LN2(x+ffn) -> out[96, 256]
"""

import os
import sys
import types

import numpy as np

sys.path.insert(0, "/opt/trn_rl_repo")

import concourse.bass as bass
import concourse.tile as tile
from concourse import bacc, mybir
from concourse.bass_utils import run_bass_kernel_spmd
from concourse.masks import make_identity

F32 = mybir.dt.float32
F32R = mybir.dt.float32r

N_CORES = 8
NFC = 256
D = 1024
KK = 25          # 5x5 kernel positions
NROI = 12        # rois per image
B = 8            # images per core
R = 96           # rows per core (B * NROI)
S = 3600         # 45*80 spatial
CIMG = 768
SHIFT = 60.0     # global softmax shift (max logit ~85; see notes)
EPS = 1e-5

NK = 29          # s-chunks of 128 (28*128 + 16)
SK_LAST = S - 128 * (NK - 1)

LAST_RESULTS = None  # BassKernelResults of the most recent run (for test.py)


def _r(x):
    return x


def build_bass():
    nc = bacc.Bacc("TRN2", target_bir_lowering=False, debug=False,
                   num_devices=N_CORES)

    # ---- DRAM I/O (per-core shard) ----
    roi_d = nc.dram_tensor("roi_t", [D, R * KK], F32R, kind="ExternalInput").ap()
    img_d = nc.dram_tensor("img_raw", [B, CIMG, S], F32R, kind="ExternalInput").ap()
    w1_d = nc.dram_tensor("w_ds1t", [D, NFC], F32R, kind="ExternalInput").ap()
    w2_d = nc.dram_tensor("w_ds2t", [CIMG, NFC], F32R, kind="ExternalInput").ap()
    we_d = nc.dram_tensor("w_embt", [NFC, KK * NFC], F32R, kind="ExternalInput").ap()
    b1_d = nc.dram_tensor("b_ds1", [NFC], F32, kind="ExternalInput").ap()
    b2_d = nc.dram_tensor("b_ds2", [NFC], F32, kind="ExternalInput").ap()
    be_d = nc.dram_tensor("b_emb", [NFC], F32, kind="ExternalInput").ap()
    fw1_d = nc.dram_tensor("ffn_w1t", [NFC, NFC], F32R, kind="ExternalInput").ap()
    fb1_d = nc.dram_tensor("ffn_b1", [NFC], F32, kind="ExternalInput").ap()
    fw2_d = nc.dram_tensor("ffn_w2t", [NFC, NFC], F32R, kind="ExternalInput").ap()
    fb2_d = nc.dram_tensor("ffn_b2", [NFC], F32, kind="ExternalInput").ap()
    g1_d = nc.dram_tensor("g1", [NFC], F32, kind="ExternalInput").ap()
    be1_d = nc.dram_tensor("be1", [NFC], F32, kind="ExternalInput").ap()
    g2_d = nc.dram_tensor("g2", [NFC], F32, kind="ExternalInput").ap()
    be2_d = nc.dram_tensor("be2", [NFC], F32, kind="ExternalInput").ap()
    out_d = nc.dram_tensor("out", [R, NFC], F32, kind="ExternalOutput").ap()

    with tile.TileContext(nc) as tc:
        _body(nc, tc, roi_d, img_d, w1_d, w2_d, we_d, b1_d, b2_d, be_d,
              fw1_d, fb1_d, fw2_d, fb2_d, g1_d, be1_d, g2_d, be2_d, out_d)

    nc.compile()
    return nc


SECS = [(0, 1024), (1024, 1024), (2048, 1024), (3072, 528)]


def _body(nc, tc, roi_d, img_d, w1_d, w2_d, we_d, b1_d, b2_d, be_d,
          fw1_d, fb1_d, fw2_d, fb2_d, g1_d, be1_d, g2_d, be2_d, out_d):
    from contextlib import ExitStack

    top = ExitStack()
    persist = top.enter_context(tc.tile_pool(name="persist", bufs=1))
    ip_w = top.enter_context(tc.tile_pool(name="iw", bufs=1))
    ip_raw = top.enter_context(tc.tile_pool(name="raw", bufs=2))
    ip_img = top.enter_context(tc.tile_pool(name="img", bufs=3))
    ps_main = top.enter_context(tc.tile_pool(name="ps_main", bufs=4,
                                             space="PSUM"))
    ps_combo = top.enter_context(tc.tile_pool(name="ps_combo", bufs=2,
                                              space="PSUM"))
    ps_ctx = top.enter_context(tc.tile_pool(name="ps_ctx", bufs=1,
                                            space="PSUM"))
    ps_emb = top.enter_context(tc.tile_pool(name="ps_emb", bufs=1,
                                            space="PSUM"))

    # --- weight / activation loads (issue order == DMA priority) ---
    w2sb = ip_w.tile([128, 6, NFC], F32R)
    nc.sync.dma_start(out=w2sb[:],
                      in_=w2_d.rearrange("(k p) o -> p k o", p=128))

    def load_raw(b, si):
        s0, sl = SECS[si]
        raw = ip_raw.tile([128, 6, 1024], F32R, tag="raw",
                          name=f"raw_{b}_{si}")
        nc.sync.dma_start(
            out=raw[:, :, :sl],
            in_=img_d[b].rearrange("(k p) s -> p k s", p=128)[
                :, :, s0:s0 + sl])
        return raw

    raw_pre = [load_raw(0, 0), load_raw(0, 1)]

    ident = persist.tile([128, 128], F32)
    make_identity(nc, ident[:])
    b2_sb = persist.tile([128, 2], F32)
    nc.sync.dma_start(out=b2_sb[:], in_=b2_d.rearrange("(c p) -> p c", p=128))
    b1_sb = persist.tile([128, 2], F32)
    nc.sync.dma_start(out=b1_sb[:], in_=b1_d.rearrange("(c p) -> p c", p=128))
    bemb_bc = persist.tile([R, NFC], F32)
    nc.sync.dma_start(out=bemb_bc[:],
                      in_=be_d[None, :].to_broadcast((R, NFC)))
    ones58 = persist.tile([128, 58], F32)
    nc.vector.memset(ones58[:], 1.0)
    nshift_sb = persist.tile([128, 1], F32)
    nc.vector.memset(nshift_sb[:], -SHIFT)
    eps_sb = persist.tile([128, 1], F32)
    nc.vector.memset(eps_sb[:], EPS)

    emb_rc = persist.tile([R, NFC], F32)      # emb in [row, o]
    embT = persist.tile([128, 2, R], F32)     # emb^T  [c_part, c_half, row]
    ctx_sb = persist.tile([R, NFC], F32)      # attention context, packed rows

    img_secs = {}

    def conv_sec(b, si):
        """img conv for section si of batch b -> img_secs[(b, si)]."""
        s0, sl = SECS[si]
        raw = raw_pre.pop(0) if raw_pre else load_raw(b, si)
        img_sb = ip_img.tile([128, 2, 1024], F32R, tag="img",
                             name=f"img_{b}_{si}")
        img_secs[(b, si)] = img_sb
        nf = 2
        fl = sl // nf
        for ch in range(2):
            pss = {}
            for Ck in range(6):
                for f in range(nf):
                    if Ck == 0:
                        pss[f] = ps_main.tile([128, 512], F32, tag="conv",
                                              name=f"cv_{b}_{si}_{ch}_{f}")
                    nc.tensor.matmul(
                        pss[f][:, :fl],
                        w2sb[:, Ck, ch * 128:(ch + 1) * 128],
                        raw[:, Ck, f * fl:(f + 1) * fl],
                        start=(Ck == 0), stop=(Ck == 5))
            for f in range(nf):
                nc.scalar.activation(
                    out=img_sb[:, ch, f * fl:(f + 1) * fl],
                    in_=pss[f][:, :fl],
                    func=mybir.ActivationFunctionType.Identity,
                    bias=b2_sb[:, ch:ch + 1], scale=1.0)

    # ==================== roi phase (streamed) ====================
    roi_scope = ExitStack()
    rp = roi_scope.enter_context(tc.tile_pool(name="roi", bufs=1))
    rpf = roi_scope.enter_context(tc.tile_pool(name="roi_f", bufs=2))
    rpg = roi_scope.enter_context(tc.tile_pool(name="roi_pg", bufs=2))

    def roi_phase_conv():
        w1sb = rp.tile([128, 8, NFC], F32R)
        nc.sync.dma_start(out=w1sb[:],
                          in_=w1_d.rearrange("(k p) o -> p k o", p=128))
        conv_roi = rp.tile([128, 2, R, KK], F32R)  # [c_part, c_half, r, p]
        roi_r = roi_d.rearrange("(k p) f -> p k f", p=128)
        for f in range(5):
            rsf = rpf.tile([128, 8, 480], F32R, tag="roisb",
                           name=f"roisb_{f}")
            nc.sync.dma_start(out=rsf[:], in_=roi_r[:, :, f * 480:(f + 1) * 480])
            for ch in range(2):
                ps = ps_main.tile([128, 480], F32, tag="conv",
                                  name=f"rcv_{ch}_{f}")
                for dk in range(8):
                    nc.tensor.matmul(
                        ps[:],
                        w1sb[:, dk, ch * 128:(ch + 1) * 128],
                        rsf[:, dk, :],
                        start=(dk == 0), stop=(dk == 7))
                nc.scalar.activation(
                    out=conv_roi[:, ch].rearrange("p r q -> p (r q)")[
                        :, f * 480:(f + 1) * 480],
                    in_=ps[:],
                    func=mybir.ActivationFunctionType.Identity,
                    bias=b1_sb[:, ch:ch + 1], scale=1.0)
        return conv_roi

    def roi_phase_emb(conv_roi):
        we_r = we_d.rearrange("(k p) f -> p k f", p=128)
        eps_ps = ps_emb.tile([R, NFC], F32, tag="emb")
        for pg in range(5):
            wes = rpg.tile([128, 2, 5, NFC], F32R, tag="wesb",
                           name=f"wesb_{pg}")
            nc.sync.dma_start(
                out=wes[:],
                in_=we_r.rearrange("p k (q o) -> p k q o", o=NFC)[
                    :, :, pg * 5:(pg + 1) * 5, :])
            for ck in range(2):
                for pl in range(5):
                    p = pg * 5 + pl
                    nc.tensor.matmul(
                        eps_ps[:],
                        conv_roi[:, ck, :, p],
                        wes[:, ck, pl, :],
                        start=(pg == 0 and ck == 0 and pl == 0),
                        stop=(pg == 4 and ck == 1 and pl == 4))
        nc.vector.tensor_add(emb_rc[:], eps_ps[:], bemb_bc[:])
        for ch in range(2):
            tps_e = ps_ctx.tile([128, R], F32, tag="ctx", name=f"embt_{ch}")
            nc.tensor.transpose(tps_e[:, :R],
                                emb_rc[:, ch * 128:(ch + 1) * 128],
                                ident[:R, :R])
            nc.vector.tensor_copy(embT[:, ch, :], tps_e[:, :R])

    # ==================== emission ====================
    conv_sec(0, 0)
    conv_sec(0, 1)
    conv_roi = roi_phase_conv()
    conv_sec(0, 2)
    roi_phase_emb(conv_roi)
    conv_sec(0, 3)
    roi_scope.close()

    # post-roi pools
    ip_imgt = top.enter_context(tc.tile_pool(name="imgt", bufs=2))
    ip_small = top.enter_context(tc.tile_pool(name="ismall", bufs=2))
    rhs_sc = [ip_w.tile([128, 140], F32R, tag=f"rhs_sc{ch}",
                        name=f"rhs_sc{ch}")
              for ch in range(2)]
    for ch in range(2):
        nc.vector.tensor_copy(rhs_sc[ch][:, 0:128], ident[:])

    def combo_sec(b, si, cps):
        """transpose+score+exp+ctx accumulation for section si of batch b."""
        img_sb = img_secs.pop((b, si))
        s0, sl = SECS[si]
        k0 = s0 // 128
        nk = (sl + 127) // 128
        imgt = ip_imgt.tile([128, 8, NFC + 2], F32R, tag="imgt",
                            name=f"imgt_{b}_{si}")
        et = ip_small.tile([128, 8, NROI], F32R, tag="et", name=f"et_{b}_{si}")
        nc.vector.tensor_copy(
            imgt[:, :nk, NFC:NFC + 2],
            ones58[:, :2 * nk].rearrange("p (k two) -> p k two", two=2))
        for j in range(nk):
            sk = min(128, sl - j * 128)
            pt = []
            for ch in range(2):
                ps = ps_combo.tile([128, 140], F32, tag="combo")
                nc.tensor.matmul(
                    ps[:sk, :],
                    img_sb[:, ch, j * 128:j * 128 + sk],
                    rhs_sc[ch][:],
                    start=True, stop=True)
                if ch == 0:
                    nc.vector.tensor_copy(
                        imgt[:sk, j, 0:128], ps[:sk, 0:128])
                else:
                    nc.scalar.activation(
                        out=imgt[:sk, j, 128:256], in_=ps[:sk, 0:128],
                        func=mybir.ActivationFunctionType.Copy,
                        bias=0.0, scale=1.0)
                pt.append(ps)
            aT = ip_small.tile([128, NROI], F32, tag="aT")
            nc.vector.tensor_copy(aT[:sk, :], pt[0][:sk, 128:140])
            aT2 = ip_small.tile([128, NROI], F32, tag="aT2")
            nc.vector.tensor_add(aT2[:sk, :], pt[1][:sk, 128:140], aT[:sk, :])
            nc.scalar.activation(
                out=et[:sk, j, :], in_=aT2[:sk, :],
                func=mybir.ActivationFunctionType.Exp,
                bias=nshift_sb[:sk, :], scale=1.0)
        for j in range(nk):
            sk = min(128, sl - j * 128)
            k = k0 + j
            nc.tensor.matmul(
                cps[:], et[:sk, j, :], imgt[:sk, j, :],
                start=(k == 0), stop=(k == NK - 1))

    def ctx_finish(b, cps):
        rinv = ip_small.tile([NROI, 1], F32, tag="rinv")
        nc.vector.reciprocal(rinv[:], cps[:, NFC:NFC + 1])
        ctx_b = ip_small.tile([NROI, NFC], F32, tag="ctx_b")
        nc.vector.tensor_scalar_mul(ctx_b[:], cps[:, 0:NFC], rinv[:])
        nc.sync.dma_start(out=ctx_sb[b * NROI:(b + 1) * NROI, :],
                          in_=ctx_b[:])

    for b in range(B):
        for ch in range(2):
            nc.vector.tensor_copy(
                rhs_sc[ch][:, 128:140],
                embT[:, ch, b * NROI:(b + 1) * NROI])
        cps = ps_ctx.tile([NROI, NFC + 2], F32, tag="ctx", name=f"ctx_{b}")
        for si in range(4):
            if b > 0:
                conv_sec(b, si)
            combo_sec(b, si, cps)
        ctx_finish(b, cps)

    # ==================== tail: LN1 -> FFN -> LN2 ====================
    with ExitStack() as t_ctx:
        tp = t_ctx.enter_context(tc.tile_pool(name="tail", bufs=1))

        def bcast(src, name):
            t = tp.tile([R, NFC], F32, name=name)
            nc.sync.dma_start(out=t[:], in_=src[None, :].to_broadcast((R, NFC)))
            return t

        g1_bc = bcast(g1_d, "g1bc")
        be1_bc = bcast(be1_d, "be1bc")
        g2_bc = bcast(g2_d, "g2bc")
        be2_bc = bcast(be2_d, "be2bc")
        fb1_sb = tp.tile([128, 2], F32)
        nc.sync.dma_start(out=fb1_sb[:],
                          in_=fb1_d.rearrange("(c p) -> p c", p=128))
        fb2_sb = tp.tile([128, 2], F32)
        nc.sync.dma_start(out=fb2_sb[:],
                          in_=fb2_d.rearrange("(c p) -> p c", p=128))
        fw1 = tp.tile([128, 2, NFC], F32R)
        nc.sync.dma_start(out=fw1[:],
                          in_=fw1_d.rearrange("(k p) o -> p k o", p=128))
        fw2 = tp.tile([128, 2, NFC], F32R)
        nc.sync.dma_start(out=fw2[:],
                          in_=fw2_d.rearrange("(k p) o -> p k o", p=128))

        def layernorm(dst, src, g_bc, b_bc):
            st = tp.tile([R, 6], F32, tag="ln_st")
            mv = tp.tile([R, 2], F32, tag="ln_mv")
            nc.vector.bn_stats(out=st[:], in_=src[:])
            nc.vector.bn_aggr(out=mv[:], in_=st[:])
            rstd = tp.tile([R, 1], F32, tag="ln_rstd")
            nc.scalar.activation(out=rstd[:], in_=mv[:, 1:2],
                                 func=mybir.ActivationFunctionType.Sqrt,
                                 bias=eps_sb[:R, :], scale=1.0)
            nc.vector.reciprocal(rstd[:], rstd[:])
            nc.vector.tensor_scalar(
                out=dst[:], in0=src[:], scalar1=mv[:, 0:1], scalar2=rstd[:],
                op0=mybir.AluOpType.subtract, op1=mybir.AluOpType.mult)
            nc.vector.tensor_mul(dst[:], dst[:], g_bc[:])
            nc.vector.tensor_add(dst[:], dst[:], b_bc[:])

        x_in = tp.tile([R, NFC], F32)
        nc.vector.tensor_add(x_in[:], ctx_sb[:], emb_rc[:])
        x = tp.tile([R, NFC], F32)
        layernorm(x, x_in, g1_bc, be1_bc)

        xT = tp.tile([128, 2, R], F32R)
        for ch in range(2):
            ps = ps_ctx.tile([128, R], F32, tag="ctx", name=f"xT_{ch}")
            nc.tensor.transpose(ps[:, :R], x[:, ch * 128:(ch + 1) * 128],
                                ident[:R, :R])
            nc.vector.tensor_copy(xT[:, ch, :], ps[:, :R])

        h = tp.tile([128, 2, R], F32R)
        for ch in range(2):
            ps = ps_combo.tile([128, R], F32, tag="combo", name=f"ffn1_{ch}")
            for ck in range(2):
                nc.tensor.matmul(ps[:, :R],
                                 fw1[:, ck, ch * 128:(ch + 1) * 128],
                                 xT[:, ck, :],
                                 start=(ck == 0), stop=(ck == 1))
            nc.scalar.activation(out=h[:, ch, :], in_=ps[:, :R],
                                 func=mybir.ActivationFunctionType.Relu,
                                 bias=fb1_sb[:, ch:ch + 1], scale=1.0)

        f_rc = tp.tile([R, NFC], F32)
        for ch in range(2):
            ps = ps_combo.tile([128, R], F32, tag="combo", name=f"ffn2_{ch}")
            for ck in range(2):
                nc.tensor.matmul(ps[:, :R],
                                 fw2[:, ck, ch * 128:(ch + 1) * 128],
                                 h[:, ck, :],
                                 start=(ck == 0), stop=(ck == 1))
            fo = tp.tile([128, R], F32, tag="ffn_o")
            nc.scalar.activation(out=fo[:], in_=ps[:, :R],
                                 func=mybir.ActivationFunctionType.Identity,
                                 bias=fb2_sb[:, ch:ch + 1], scale=1.0)
            pst = ps_ctx.tile([R, 128], F32, tag="ctx", name=f"fT_{ch}")
            nc.tensor.transpose(pst[:], fo[:], ident[:])
            nc.vector.tensor_copy(f_rc[:, ch * 128:(ch + 1) * 128], pst[:])

        x2 = tp.tile([R, NFC], F32)
        nc.vector.tensor_add(x2[:], x[:], f_rc[:])
        out_sb = tp.tile([R, NFC], F32)
        layernorm(out_sb, x2, g2_bc, be2_bc)
        nc.sync.dma_start(out=out_d[:], in_=out_sb[:])

    top.close()


_NC_CACHE = None


def _get_nc():
    global _NC_CACHE
    if _NC_CACHE is None:
        _NC_CACHE = build_bass()
    return _NC_CACHE


def _install_ntff_hook():
    """The image's antenv lacks axon_hooks; register the NTFF profile hook."""
    if "antenv.axon_hooks" in sys.modules:
        return
    try:
        sys.path.insert(0, "/root/.axon_site/trn_agent_boot")
        import trn_boot
        hook = trn_boot._ntff_profile_via_ctypes("/opt/axon/libaxon_pjrt.so")
        m = types.ModuleType("antenv.axon_hooks")
        m.get_axon_ntff_profile_hook = lambda: hook
        sys.modules["antenv.axon_hooks"] = m
    except Exception:
        pass


def prepare_in_maps(roi_feature, image_feature, w_ds1, b_ds1, w_ds2, b_ds2,
                    w_emb, b_emb, g1, be1, ffn_w1, ffn_b1, ffn_w2, ffn_b2,
                    g2, be2, layer_id=1, **_unused):
    roi_feature = np.ascontiguousarray(np.asarray(roi_feature, dtype=np.float32))
    image_feature = np.ascontiguousarray(np.asarray(image_feature, dtype=np.float32))

    f32 = lambda x: np.ascontiguousarray(np.asarray(x, dtype=np.float32))
    w_ds1t = f32(w_ds1).T.copy()                       # [D, NFC]
    w_ds2t = f32(w_ds2).T.copy()                       # [CIMG, NFC]
    w_embt = np.ascontiguousarray(
        f32(w_emb).reshape(NFC, NFC, KK).transpose(1, 2, 0).reshape(
            NFC, KK * NFC))                            # [c, p*o]
    ffn_w1t = f32(ffn_w1).T.copy()
    ffn_w2t = f32(ffn_w2).T.copy()

    shared = {
        "w_ds1t": w_ds1t, "w_ds2t": w_ds2t, "w_embt": w_embt,
        "b_ds1": f32(b_ds1), "b_ds2": f32(b_ds2), "b_emb": f32(b_emb),
        "ffn_w1t": ffn_w1t, "ffn_b1": f32(ffn_b1),
        "ffn_w2t": ffn_w2t, "ffn_b2": f32(ffn_b2),
        "g1": f32(g1), "be1": f32(be1), "g2": f32(g2), "be2": f32(be2),
    }

    roi_r = roi_feature.reshape(N_CORES, R, D, KK)
    img_r = image_feature.reshape(N_CORES, B, CIMG, S)
    in_maps = []
    for i in range(N_CORES):
        roi_t = np.ascontiguousarray(
            roi_r[i].transpose(1, 0, 2)).reshape(D, R * KK)
        in_maps.append({"roi_t": roi_t,
                        "img_raw": np.ascontiguousarray(img_r[i]),
                        **shared})
    return in_maps


def kernel(**inputs):
    global LAST_RESULTS
    in_maps = prepare_in_maps(**inputs)
    nc = _get_nc()
    trace = os.environ.get("BASS_KERNEL_TRACE", "0") == "1"
    if trace:
        _install_ntff_hook()
    LAST_RESULTS = run_bass_kernel_spmd(
        nc, in_maps, list(range(N_CORES)), trace=trace)
    out = np.concatenate([LAST_RESULTS.results[i]["out"]
                          for i in range(N_CORES)], axis=0)
    return out
